# revision 2
# baseline (speedup 1.0000x reference)
"""Trainium2 Bass kernel for vertices_to_edges (gnn_message_passing).

out[b, c, e] = 0.5 * (VT[b, edges[b,e,0], c] + VT[b, edges[b,e,1], c])

Sharding: B=4 batches x 2 edge-halves -> 8 cores (data parallel; each core
holds one batch's channel-padded bf16 vertex table in DRAM).

Per core, endpoint rows are pulled with the GPSIMD `dma_gather` custom
instruction in TRANSPOSE mode on a bf16 table (rows of 128 bf16 = 256B):
the DMA crossbar deposits gathered rows channels-first ([128 ch, n_idx]),
so no PE transpose / PSUM stage is needed. The vertex table is pre-scaled
by 0.5 on the host, so the device compute is a single DVE tensor_add per
tile, written out as bf16 (host widens to f32 during unshard).

All transpose gathers are issued on ONE SWDGE queue: the DMA engines
round-robin *between* queues at packet granularity, and interleaving two
transpose gathers on one engine corrupts the shared xbar transposer state
(empirically verified); same-queue gathers drain in ring order and are
safe. Output writes go through HWDGE (sync engine) and may interleave
freely.

To fit V=150000 into int16 indices, edges are lex-sorted by
(chunk(v1), chunk(v2)) with 32768-row chunks: every run gathers both
endpoints with chunk-local indices against a base-offset table slice.
Runs are padded to 128-slot multiples (shared sizes across all 8 cores so
one SPMD program serves all). The host folds the sort permutation back
during unshard (index bookkeeping only).
"""

import numpy as np

B, V, E, C = 4, 150000, 450000, 62
CPH = 128  # bf16 channels per row: 256B
P = 128
N_CORES = 8
EH = E // 2  # 225000 edges per core
CHUNK_SHIFT = 15
CHUNK = 1 << CHUNK_SHIFT  # 32768
NCH = (V + CHUNK - 1) // CHUNK  # 5
TILE_E = 4096

_CACHE = {}


def _plan(run_pad):
    """run_pad: [NCH*NCH] shared padded run sizes (multiples of 128).
    Returns (runs, s_pad, g1_calls, g2_calls, n_tiles)."""
    runs = []
    s = 0
    for a in range(NCH):
        for b in range(NCH):
            n = int(run_pad[a * NCH + b])
            if n:
                runs.append([a, b, s, s + n])
                s += n
    s_pad = ((s + TILE_E - 1) // TILE_E) * TILE_E
    if s_pad > s:
        runs[-1][3] = s_pad  # extend last run with pad slots
    n_tiles = s_pad // TILE_E

    # g1 spans: consecutive runs share `a`
    spans = []
    for a, b, s0, s1 in runs:
        if spans and spans[-1][0] == a:
            spans[-1][2] = s1
        else:
            spans.append([a, s0, s1])

    MAX_IDX = 4096

    def intersect(items, t):
        t0, t1 = t * TILE_E, (t + 1) * TILE_E
        out = []
        for base_chunk, s0, s1 in items:
            lo, hi = max(s0, t0), min(s1, t1)
            while lo < hi:
                mid = min(lo + MAX_IDX, hi)
                out.append((lo, mid, base_chunk))
                lo = mid
        return out

    g1_calls = [intersect(spans, t) for t in range(n_tiles)]
    g2_calls = [intersect([(b, s0, s1) for a, b, s0, s1 in runs], t) for t in range(n_tiles)]
    return runs, s_pad, g1_calls, g2_calls, n_tiles


def _build_module(s_pad, g1_calls, g2_calls, n_tiles, reps=1):
    import concourse.bass as bass
    import concourse.tile as tile
    from concourse import bacc, mybir

    nc = bacc.Bacc("TRN2", target_bir_lowering=False, debug=False, num_devices=N_CORES, num_swdge_queues=4)
    table = nc.dram_tensor("table", [V, CPH], mybir.dt.bfloat16, kind="ExternalInput")
    i1 = nc.dram_tensor("i1", [128, s_pad // 16], mybir.dt.int16, kind="ExternalInput")
    i2 = nc.dram_tensor("i2", [128, s_pad // 16], mybir.dt.int16, kind="ExternalInput")
    out = nc.dram_tensor("out", [C, s_pad], mybir.dt.bfloat16, kind="ExternalOutput")

    def rows_of(chunk):
        return min(CHUNK, V - chunk * CHUNK)

    with tile.TileContext(nc) as tc:
        with (
            tc.tile_pool(name="idx", bufs=1) as idxp,
            tc.tile_pool(name="gat", bufs=4) as gatp,
            tc.tile_pool(name="outp", bufs=3) as outp,
        ):
            i1_sb = idxp.tile([128, s_pad // 16], mybir.dt.int16)
            i2_sb = idxp.tile([128, s_pad // 16], mybir.dt.int16)
            nc.sync.dma_start(i1_sb[:], i1.ap())
            nc.sync.dma_start(i2_sb[:], i2.ap())

            for _ in range(reps):
                for t in range(n_tiles):
                    g1 = gatp.tile([P, 1, TILE_E], mybir.dt.bfloat16, tag="g1")
                    g2 = gatp.tile([P, 1, TILE_E], mybir.dt.bfloat16, tag="g2")
                    for g, calls, isb in ((g1, g1_calls[t], i1_sb), (g2, g2_calls[t], i2_sb)):
                        for s0, s1, chunk in calls:
                            c0 = s0 - t * TILE_E
                            c1 = s1 - t * TILE_E
                            n = s1 - s0
                            nc.gpsimd.dma_gather(
                                out_ap=g[:, :, c0:c1],
                                in_ap=table.ap()[chunk * CHUNK : chunk * CHUNK + rows_of(chunk), :],
                                idxs_ap=isb[:, s0 // 16 : s1 // 16],
                                num_idxs=n,
                                num_idxs_reg=n,
                                elem_size=CPH,
                                transpose=True,
                                single_packet=False,
                                queue_num=0,
                            )
                    o = outp.tile([C, TILE_E], mybir.dt.bfloat16, tag="o")
                    nc.vector.tensor_add(o[:], g1[0:C, 0, :], g2[0:C, 0, :])
                    nc.sync.dma_start(out.ap()[:, t * TILE_E : (t + 1) * TILE_E], o[:])

    nc.compile()
    return nc


def _wrap16_rep(flat_i16):
    w = np.ascontiguousarray(flat_i16.reshape(-1, 16).T)  # [16, S/16]
    # replicated for every 16-partition Q7 window (tx/rx cpu pairs x queues)
    return np.ascontiguousarray(np.tile(w, (8, 1)))


LAST_RESULT = None


def _prepare(inputs, reps=1):
    import ml_dtypes

    vertex_tokens = np.asarray(inputs["vertex_tokens"], dtype=np.float32)
    edges = np.asarray(inputs["edges"]).astype(np.int32)

    # host prep: per-core lex-sort by (chunk(v1), chunk(v2))
    cores = []
    counts_all = np.zeros((N_CORES, NCH * NCH), dtype=np.int64)
    for core in range(N_CORES):
        b, half = divmod(core, 2)
        ed = edges[b, half * EH : (half + 1) * EH]
        v1, v2 = ed[:, 0], ed[:, 1]
        key = (v1 >> CHUNK_SHIFT) * NCH + (v2 >> CHUNK_SHIFT)
        order = np.argsort(key, kind="stable").astype(np.int32)
        counts_all[core] = np.bincount(key, minlength=NCH * NCH)
        cores.append((v1, v2, key, order))

    run_pad = ((counts_all.max(axis=0) + P - 1) // P) * P
    runs, s_pad, g1_calls, g2_calls, n_tiles = _plan(run_pad)

    cache_key = (s_pad, str(g1_calls), str(g2_calls), reps)
    if cache_key not in _CACHE:
        _CACHE.clear()
        _CACHE[cache_key] = _build_module(s_pad, g1_calls, g2_calls, n_tiles, reps=reps)
    nc = _CACHE[cache_key]

    # bf16 table pre-scaled by 0.5 (folds the averaging into the gather data)
    table_pad = np.zeros((B, V, CPH), dtype=ml_dtypes.bfloat16)
    table_pad[:, :, :C] = (vertex_tokens * 0.5).astype(ml_dtypes.bfloat16)

    in_maps = []
    eslots = []
    for core in range(N_CORES):
        v1, v2, key, order = cores[core]
        counts = counts_all[core]
        idx1 = np.zeros(s_pad, dtype=np.int16)
        idx2 = np.zeros(s_pad, dtype=np.int16)
        eslot = np.full(s_pad, -1, dtype=np.int32)
        pos = 0
        for a, bb, s0, s1 in runs:
            n = int(counts[a * NCH + bb])
            seg = order[pos : pos + n]
            pos += n
            idx1[s0 : s0 + n] = (v1[seg] - (a << CHUNK_SHIFT)).astype(np.int16)
            idx2[s0 : s0 + n] = (v2[seg] - (bb << CHUNK_SHIFT)).astype(np.int16)
            eslot[s0 : s0 + n] = seg
        b, half = divmod(core, 2)
        in_maps.append(
            {
                "table": table_pad[b],
                "i1": _wrap16_rep(idx1),
                "i2": _wrap16_rep(idx2),
            }
        )
        eslots.append(eslot)

    return nc, in_maps, eslots


def _unshard(results, eslots):
    out_ec = np.empty((B, E, C), dtype=np.float32)
    for core in range(N_CORES):
        b, half = divmod(core, 2)
        eslot = eslots[core]
        valid = eslot >= 0
        col_of_edge = np.empty(EH, dtype=np.int64)
        col_of_edge[eslot[valid]] = np.flatnonzero(valid)
        dev = np.asarray(results[core]["out"])  # [C, s_pad] bf16
        # widen bf16 -> f32 via bit trick (fast, exact)
        dev_f32 = (dev.view(np.uint16).astype(np.uint32) << 16).view(np.float32)
        out_ec[b, half * EH : (half + 1) * EH, :] = dev_f32.T[col_of_edge]
    return out_ec.transpose(0, 2, 1)


def kernel(**inputs) -> np.ndarray:
    global LAST_RESULT
    from concourse.bass_utils import run_bass_kernel_spmd

    nc, in_maps, eslots = _prepare(inputs)
    res = run_bass_kernel_spmd(nc, in_maps, core_ids=list(range(N_CORES)))
    LAST_RESULT = res
    return _unshard(res.results, eslots)


# revision 3
# speedup vs baseline: 1.9444x; 1.9444x over previous
"""Trainium2 Bass kernel for vertices_to_edges (gnn_message_passing).

out[b, c, e] = 0.5 * (VT[b, edges[b,e,0], c] + VT[b, edges[b,e,1], c])

Sharding: B=4 batches x 2 edge-halves -> 8 cores (data parallel; each core
holds one batch's vertex table in DRAM).

The SWDGE dma_gather path is descriptor-rate limited: 256B descriptors run
at ~93 GB/s across 4 queues while 512B descriptors run at ~300 GB/s
(measured). So the vertex table is stored with each row DUPLICATED
(row v = [0.5*feat_v | 0.5*feat_v], 128 f32 = 512B): per-edge gathers move
2x the bytes but at 3.3x the descriptor rate -> ~1.6x faster overall.

Per core, per 4096-edge tile:
  - 2 x dma_gather (non-transpose, f32, elem 512B, queues round-robin 0-3)
    pull endpoint rows into [128, 32, 128] tiles (edge slot s -> partition
    s%128, free slot s//128),
  - one DVE tensor_add over the strided [128, 32, 0:64] halves sums the
    endpoints (the 0.5 scaling is folded into the table) straight to a
    bf16 [128, 32*64] tile,
  - HWDGE writes it to a partition-major DRAM layout [128, s_pad/128*64];
    the host untangles slot->edge order and widens bf16->f32 during
    unshard (index bookkeeping only).

NOTE: transpose-mode dma_gather (which would deposit channels-first
directly) is NOT used: concurrent transpose gathers on different queues
corrupt each other via the shared DMA xbar transposer (verified on HW),
and a single queue is limited to ~35 GB/s.

To fit V=150000 into int16 indices, edges are lex-sorted by
(chunk(v1), chunk(v2)) with 32768-row chunks: every run gathers both
endpoints with chunk-local indices against a base-offset table slice.
Runs are padded to 128-slot multiples (shared sizes across all 8 cores so
one SPMD program serves all).
"""

import numpy as np

B, V, E, C = 4, 150000, 450000, 62
CP = 64  # f32 channels per half-row
CPH = 128  # duplicated row: 128 f32 = 512B
P = 128
N_CORES = 8
EH = E // 2  # 225000 edges per core
CHUNK_SHIFT = 15
CHUNK = 1 << CHUNK_SHIFT  # 32768
NCH = (V + CHUNK - 1) // CHUNK  # 5
TILE_E = 4096
K = TILE_E // P  # 32 slots per partition per tile

_CACHE = {}


def _plan(run_pad):
    """run_pad: [NCH*NCH] shared padded run sizes (multiples of 128).
    Returns (runs, s_pad, g1_calls, g2_calls, n_tiles)."""
    runs = []
    s = 0
    for a in range(NCH):
        for b in range(NCH):
            n = int(run_pad[a * NCH + b])
            if n:
                runs.append([a, b, s, s + n])
                s += n
    s_pad = ((s + TILE_E - 1) // TILE_E) * TILE_E
    if s_pad > s:
        runs[-1][3] = s_pad  # extend last run with pad slots
    n_tiles = s_pad // TILE_E

    # g1 spans: consecutive runs share `a`
    spans = []
    for a, b, s0, s1 in runs:
        if spans and spans[-1][0] == a:
            spans[-1][2] = s1
        else:
            spans.append([a, s0, s1])

    MAX_IDX = 4096

    def intersect(items, t):
        t0, t1 = t * TILE_E, (t + 1) * TILE_E
        out = []
        for base_chunk, s0, s1 in items:
            lo, hi = max(s0, t0), min(s1, t1)
            while lo < hi:
                mid = min(lo + MAX_IDX, hi)
                out.append((lo, mid, base_chunk))
                lo = mid
        return out

    g1_calls = [intersect(spans, t) for t in range(n_tiles)]
    g2_calls = [intersect([(b, s0, s1) for a, b, s0, s1 in runs], t) for t in range(n_tiles)]
    return runs, s_pad, g1_calls, g2_calls, n_tiles


def _build_module(s_pad, g1_calls, g2_calls, n_tiles, reps=1):
    import concourse.bass as bass
    import concourse.tile as tile
    from concourse import bacc, mybir

    nc = bacc.Bacc("TRN2", target_bir_lowering=False, debug=False, num_devices=N_CORES, num_swdge_queues=4)
    table = nc.dram_tensor("table", [V, CPH], mybir.dt.float32, kind="ExternalInput")
    i1 = nc.dram_tensor("i1", [128, s_pad // 16], mybir.dt.int16, kind="ExternalInput")
    i2 = nc.dram_tensor("i2", [128, s_pad // 16], mybir.dt.int16, kind="ExternalInput")
    out = nc.dram_tensor("out", [P, (s_pad // P) * CP], mybir.dt.bfloat16, kind="ExternalOutput")

    def rows_of(chunk):
        return min(CHUNK, V - chunk * CHUNK)

    qctr = [0]
    with tile.TileContext(nc) as tc:
        with (
            tc.tile_pool(name="idx", bufs=1) as idxp,
            tc.tile_pool(name="gat", bufs=2) as gatp,
            tc.tile_pool(name="outp", bufs=3) as outp,
        ):
            i1_sb = idxp.tile([128, s_pad // 16], mybir.dt.int16)
            i2_sb = idxp.tile([128, s_pad // 16], mybir.dt.int16)
            nc.sync.dma_start(i1_sb[:], i1.ap())
            nc.sync.dma_start(i2_sb[:], i2.ap())

            for _ in range(reps):
                for t in range(n_tiles):
                    g1 = gatp.tile([P, K, CPH], mybir.dt.float32, tag="g1")
                    g2 = gatp.tile([P, K, CPH], mybir.dt.float32, tag="g2")
                    for g, calls, isb in ((g1, g1_calls[t], i1_sb), (g2, g2_calls[t], i2_sb)):
                        for s0, s1, chunk in calls:
                            k0 = (s0 - t * TILE_E) // P
                            k1 = (s1 - t * TILE_E) // P
                            n = s1 - s0
                            nc.gpsimd.dma_gather(
                                out_ap=g[:, k0:k1, :],
                                in_ap=table.ap()[chunk * CHUNK : chunk * CHUNK + rows_of(chunk), :],
                                idxs_ap=isb[:, s0 // 16 : s1 // 16],
                                num_idxs=n,
                                num_idxs_reg=n,
                                elem_size=CPH,
                                transpose=False,
                                single_packet=False,
                                queue_num=qctr[0] % 4,
                            )
                            qctr[0] += 1
                    o = outp.tile([P, K, CP], mybir.dt.bfloat16, tag="o")
                    nc.vector.tensor_add(o[:], g1[:, :, 0:CP], g2[:, :, 0:CP])
                    nc.sync.dma_start(out.ap()[:, t * K * CP : (t + 1) * K * CP], o[:])

    nc.compile()
    return nc


def _wrap16_rep(flat_i16):
    w = np.ascontiguousarray(flat_i16.reshape(-1, 16).T)  # [16, S/16]
    # replicated for every 16-partition Q7 window (tx/rx cpu pairs x queues)
    return np.ascontiguousarray(np.tile(w, (8, 1)))


LAST_RESULT = None


def _prepare(inputs, reps=1):
    vertex_tokens = np.asarray(inputs["vertex_tokens"], dtype=np.float32)
    edges = np.asarray(inputs["edges"]).astype(np.int32)

    # host prep: per-core lex-sort by (chunk(v1), chunk(v2))
    cores = []
    counts_all = np.zeros((N_CORES, NCH * NCH), dtype=np.int64)
    for core in range(N_CORES):
        b, half = divmod(core, 2)
        ed = edges[b, half * EH : (half + 1) * EH]
        v1, v2 = ed[:, 0], ed[:, 1]
        key = (v1 >> CHUNK_SHIFT) * NCH + (v2 >> CHUNK_SHIFT)
        order = np.argsort(key, kind="stable").astype(np.int32)
        counts_all[core] = np.bincount(key, minlength=NCH * NCH)
        cores.append((v1, v2, key, order))

    run_pad = ((counts_all.max(axis=0) + P - 1) // P) * P
    runs, s_pad, g1_calls, g2_calls, n_tiles = _plan(run_pad)

    cache_key = (s_pad, str(g1_calls), str(g2_calls), reps)
    if cache_key not in _CACHE:
        _CACHE.clear()
        _CACHE[cache_key] = _build_module(s_pad, g1_calls, g2_calls, n_tiles, reps=reps)
    nc = _CACHE[cache_key]

    # f32 table, rows duplicated (512B descriptors), pre-scaled by 0.5
    half = np.zeros((B, V, CP), dtype=np.float32)
    half[:, :, :C] = vertex_tokens * 0.5
    table_pad = np.concatenate([half, half], axis=2)  # [B, V, 128]

    in_maps = []
    eslots = []
    for core in range(N_CORES):
        v1, v2, key, order = cores[core]
        counts = counts_all[core]
        idx1 = np.zeros(s_pad, dtype=np.int16)
        idx2 = np.zeros(s_pad, dtype=np.int16)
        eslot = np.full(s_pad, -1, dtype=np.int32)
        pos = 0
        for a, bb, s0, s1 in runs:
            n = int(counts[a * NCH + bb])
            seg = order[pos : pos + n]
            pos += n
            idx1[s0 : s0 + n] = (v1[seg] - (a << CHUNK_SHIFT)).astype(np.int16)
            idx2[s0 : s0 + n] = (v2[seg] - (bb << CHUNK_SHIFT)).astype(np.int16)
            eslot[s0 : s0 + n] = seg
        b, half_i = divmod(core, 2)
        in_maps.append(
            {
                "table": table_pad[b],
                "i1": _wrap16_rep(idx1),
                "i2": _wrap16_rep(idx2),
            }
        )
        eslots.append(eslot)

    return nc, in_maps, eslots


def _unshard(results, eslots):
    out_ec = np.empty((B, E, C), dtype=np.float32)
    for core in range(N_CORES):
        b, half = divmod(core, 2)
        eslot = eslots[core]
        s_pad = eslot.shape[0]
        valid = eslot >= 0
        slot_of_edge = np.empty(EH, dtype=np.int64)
        slot_of_edge[eslot[valid]] = np.flatnonzero(valid)
        dev = np.asarray(results[core]["out"]).reshape(P, s_pad // P, CP)
        # edge slot s lives at dev[s % 128, s // 128, :]
        rows = dev[slot_of_edge % P, slot_of_edge // P, :C]  # [EH, C] bf16
        out_ec[b, half * EH : (half + 1) * EH, :] = (
            rows.view(np.uint16).astype(np.uint32) << 16
        ).view(np.float32)
    return out_ec.transpose(0, 2, 1)


def kernel(**inputs) -> np.ndarray:
    global LAST_RESULT
    from concourse.bass_utils import run_bass_kernel_spmd

    nc, in_maps, eslots = _prepare(inputs)
    res = run_bass_kernel_spmd(nc, in_maps, core_ids=list(range(N_CORES)))
    LAST_RESULT = res
    return _unshard(res.results, eslots)
